# revision 13
# baseline (speedup 1.0000x reference)
"""Trainium2 kernel for nn_CosinePairwiseLoss.

Math: for unit-normalized rows f_i and class labels pred_i, the reference
computes   loss = 1 - mean_c [ (sum_{i<j, both in c} f_i.f_j) / C(n_c,2) ].
Since sum_{i!=j in c} f_i.f_j = ||S_c||^2 - n_c with S_c = sum_{i in c} f_i,
the whole problem reduces to a per-class segment-sum of normalized rows
(C x D) plus counts — O(N*D) memory-bound work, no N x N similarity matrix.

Device kernel (per core, rows sharded 8 ways; HAND-SYNCED, no TileContext —
saves the ~650ns entry barrier, the exit double-barrier, and the Tile
scheduler's reorderings):
  - 2 input DMA chunks [128, 8x64] bf16 (pred rides chunk1 as bitcast f32).
  - DVE: squares (TT mult, 2x mode) + row-sum (TensorReduce) per chunk.
  - ACT: Abs_reciprocal_sqrt -> 1/||row||  (a dummy act first makes the
    compiler's table load run at t~50, fully hidden under the input DMA).
  - DVE: per row-group g, ONE fused tensor_scalar
        ohn[p,c] = (iota[c] == pred[p,g]) * rnorm[p,g]
    (is_equal+mult in 4x DVE mode, 77ns) — the onehot pre-scaled by 1/norm,
    so PE's 16 accumulating matmuls produce the normalized segment sums
    directly; no separate normalize pass.
  - Output: PSUM -> SBUF copy (x dsq=1.0 keeps the dummy act live), then a
    SWDGE dma_scatter_add whose descriptors were PREPARED on Pool at t~700
    (hidden under the DMA wait) and are fired by trigger_dma — skipping the
    625ns HWDGE issue + 650ns DGE delay a normal DMA pays on the tail.
    Scatter-add needs zeroed DRAM: an early off-critical-path DMA of a zero
    tile handles that.

Host sums the 8 partial S matrices, adds counts (bincount), and finishes the
O(C) scalar math.
"""

import numpy as np

N, D, C = 16384, 64, 64
NCORES = 8
ROWS = N // NCORES  # 2048 rows per core
P = 128             # SBUF partitions
NT = ROWS // P      # 16 row groups per partition
CH = 2              # DMA/compute chunks
NPC = NT // CH      # row groups per chunk

# comb layout per partition, in bf16 slots:
#   [0 : 512)      feature groups 0-7   (chunk0)
#   [512 : 1024)   feature groups 8-15  (chunk1)
#   [1024 : 1056)  pred as f32x16 (bitcast view), rides chunk1
SLOTS = NT * D + 2 * NT  # 1056

_NC_CACHE = {}


def _build_nc():
    import concourse.mybir as mybir
    from concourse import bacc

    f32 = mybir.dt.float32
    bf16 = mybir.dt.bfloat16
    i16 = mybir.dt.int16
    Alu = mybir.AluOpType
    Act = mybir.ActivationFunctionType

    nc = bacc.Bacc("TRN2", target_bir_lowering=False, debug=False)

    comb_d = nc.dram_tensor("comb", [P, SLOTS], bf16, kind="ExternalInput")
    out_d = nc.dram_tensor("out", [C, D], f32, kind="ExternalOutput")

    # SBUF / PSUM
    comb0 = nc.alloc_sbuf_tensor("comb0", [P, NPC, D], bf16)
    comb1 = nc.alloc_sbuf_tensor("comb1", [P, NPC * D + 2 * NT], bf16)
    scr0 = nc.alloc_sbuf_tensor("scr0", [P, NPC, D], bf16)
    scr1 = nc.alloc_sbuf_tensor("scr1", [P, NPC, D], bf16)
    nsq = [nc.alloc_sbuf_tensor(f"nsq{k}", [P, NPC], f32) for k in range(CH)]
    rnorm = [nc.alloc_sbuf_tensor(f"rn{k}", [P, NPC], f32) for k in range(CH)]
    iot = nc.alloc_sbuf_tensor("iot", [P, C], bf16)
    idxs = nc.alloc_sbuf_tensor("idxs", [P, C // 16], i16)
    zc = nc.alloc_sbuf_tensor("zc", [C, 1], f32)
    dsq = nc.alloc_sbuf_tensor("dsq", [C, 1], f32)
    zeros = nc.alloc_sbuf_tensor("zeros", [C, D], f32)
    res = nc.alloc_sbuf_tensor("res", [P, 1, D], f32)
    ohn = [nc.alloc_sbuf_tensor(f"ohn{g}", [P, C], bf16) for g in range(NT)]
    acc = nc.alloc_psum_tensor("acc", [C, D], f32)

    # semaphores
    sem = {
        n: nc.alloc_semaphore(n)
        for n in (
            "s_in0", "s_in1", "s_zero", "s_zmem", "s_zc", "s_iot", "s_resm",
            "s_nsq0", "s_nsq1", "s_rn0", "s_rn1", "s_ohn", "s_mml", "s_res",
            "s_scat", "s_prep", "s_idx", "s_dve", "s_dsq",
        )
    }

    fch = [comb0[:], comb1[:, 0 : NPC * D].rearrange("p (j d) -> p j d", d=D)]

    # ---- SP: input DMAs, then the output-zeroing DMA ----
    # (no manual sem_clears: the Bacc preamble range-clears all semaphores)
    nc.sync.dma_start(
        comb0[:], comb_d[:, 0 : NPC * D].rearrange("p (j d) -> p j d", d=D)
    ).then_inc(sem["s_in0"], 16)
    nc.sync.dma_start(comb1[:], comb_d[:, NPC * D : SLOTS]).then_inc(
        sem["s_in1"], 16
    )
    nc.sync.wait_ge(sem["s_zmem"], 1)
    nc.sync.dma_start(out_d[:], zeros[:]).then_inc(sem["s_zero"], 16)

    # ---- ACT: dummy act (pulls the table load to t~50), then the rsqrts ----
    nc.scalar.wait_ge(sem["s_zc"], 1)
    nc.scalar.activation(dsq[:], zc[:], Act.Abs_reciprocal_sqrt).then_inc(
        sem["s_dsq"], 1
    )
    for k in range(CH):
        nc.scalar.wait_ge(sem[f"s_nsq{k}"], 1)
        nc.scalar.activation(
            rnorm[k][:], nsq[k][:], Act.Abs_reciprocal_sqrt
        ).then_inc(sem[f"s_rn{k}"], 1)

    # ---- Pool: constants, scatter prep early; trigger at the very end ----
    nc.gpsimd.memset(zc[:], 1.0).then_inc(sem["s_zc"], 1)
    nc.gpsimd.memset(zeros[:], 0.0).then_inc(sem["s_zmem"], 1)
    nc.gpsimd.iota(
        iot[:], pattern=[[1, C]], base=0, channel_multiplier=0,
        allow_small_or_imprecise_dtypes=True,
    ).then_inc(sem["s_iot"], 1)
    # scatter index i lives at [i % 16, i // 16] -> idx[p, s] = 16s + p.
    # Rows 16-127 are never read by the scatter but must hold in-range
    # values for the interpreter's bounds check -> memset 0 first.
    nc.gpsimd.memset(idxs[:], 0).then_inc(sem["s_idx"], 1)
    nc.gpsimd.wait_ge(sem["s_idx"], 1)
    nc.gpsimd.iota(
        idxs[0:16, :], pattern=[[16, C // 16]], base=0, channel_multiplier=1,
        allow_small_or_imprecise_dtypes=True,
    ).then_inc(sem["s_idx"], 1)
    nc.gpsimd.memset(res[:], 0.0).then_inc(sem["s_resm"], 1)
    # SWDGE descriptor generation (~1.1us) — hidden under the input DMA wait.
    # Descriptors encode idxs and SBUF/DRAM addresses; res data is read at
    # trigger time.
    nc.gpsimd.wait_ge(sem["s_idx"], 2)
    nc.gpsimd.dma_scatter_add(
        out_ap=out_d[:],
        in_ap=res[:],
        idxs_ap=idxs[:],
        num_idxs=C,
        num_idxs_reg=C,
        elem_size=D,
        prepare_only=True,
        sem=sem["s_scat"],
    ).then_inc(sem["s_prep"], 1)
    nc.gpsimd.wait_ge(sem["s_prep"], 1)   # Q7 desc-gen committed to the ring
    nc.gpsimd.wait_ge(sem["s_zero"], 16)  # DRAM out zeroed
    nc.gpsimd.wait_ge(sem["s_res"], 1)    # res written
    nc.gpsimd.trigger_dma(count=1)
    nc.gpsimd.wait_ge(sem["s_scat"], 16)  # scatter landed; gates kernel end

    # ---- DVE: norms, fused scaled-onehots, final PSUM->SBUF copy ----
    for k in range(CH):
        nc.vector.wait_ge(sem[f"s_in{k}"], 16)
        scr = (scr0, scr1)[k]
        nc.vector.tensor_mul(scr[:], fch[k], fch[k]).then_inc(sem["s_dve"], 1)
        nc.vector.wait_ge(sem["s_dve"], k + 1)
        nc.vector.tensor_reduce(
            nsq[k][:], scr[:], axis=mybir.AxisListType.X, op=Alu.add
        ).then_inc(sem[f"s_nsq{k}"], 1)
    nc.vector.wait_ge(sem["s_iot"], 1)
    for g in range(NT):
        k, j = divmod(g, NPC)
        if j == 0:
            nc.vector.wait_ge(sem[f"s_rn{k}"], 1)
        predf_g = comb1[:, NPC * D + 2 * g : NPC * D + 2 * (g + 1)].bitcast(f32)
        nc.vector.tensor_scalar(
            ohn[g][:], iot[:], predf_g, rnorm[k][:, j : j + 1],
            Alu.is_equal, Alu.mult,
        ).then_inc(sem["s_ohn"], 1)
    nc.vector.wait_ge(sem["s_mml"], 1)
    nc.vector.wait_ge(sem["s_resm"], 1)
    nc.vector.wait_ge(sem["s_dsq"], 1)
    nc.vector.tensor_scalar(
        res[0:C, 0, :], acc[:], dsq[:, 0:1], None, Alu.mult
    ).then_inc(sem["s_res"], 1)

    # ---- PE: 16 accumulating matmuls ----
    nc.tensor.wait_ge(sem["s_in0"], 16)
    for g in range(NT):
        k, j = divmod(g, NPC)
        if g == NPC:
            nc.tensor.wait_ge(sem["s_in1"], 16)
        nc.tensor.wait_ge(sem["s_ohn"], g + 1)
        mm = nc.tensor.matmul(
            acc[:], ohn[g][:], fch[k][:, j, :],
            start=(g == 0), stop=(g == NT - 1),
        )
    mm.then_inc(sem["s_mml"], 1)

    nc.compile()
    return nc


def _get_nc():
    if "nc" not in _NC_CACHE:
        _NC_CACHE["nc"] = _build_nc()
    return _NC_CACHE["nc"]


def _make_in_maps(feature, pred):
    import ml_dtypes

    feature = np.asarray(feature).astype(ml_dtypes.bfloat16)
    pred_f32 = np.asarray(pred).astype(np.float32)
    in_maps = []
    for c in range(NCORES):
        comb = np.zeros((P, SLOTS), np.uint16)
        fs = feature[c * ROWS : (c + 1) * ROWS].reshape(P, NT * D)
        comb[:, 0 : NT * D] = fs.view(np.uint16)
        ps_ = pred_f32[c * ROWS : (c + 1) * ROWS].reshape(P, NT)
        comb[:, NT * D : SLOTS] = ps_.view(np.uint16).reshape(P, 2 * NT)
        in_maps.append({"comb": comb.view(ml_dtypes.bfloat16)})
    return in_maps


def _finish(partials, pred):
    """Combine per-core partial segment sums into the scalar loss."""
    pred_i = np.asarray(pred).astype(np.int64)
    S = np.zeros((C, D), np.float64)
    for p in partials:
        S += p.astype(np.float64)
    counts = np.bincount(pred_i, minlength=C).astype(np.float64)
    cls_pair_sum = 0.5 * ((S * S).sum(axis=1) - counts)
    pair_counts = counts * (counts - 1.0) * 0.5
    avg = np.where(pair_counts > 0, cls_pair_sum / np.maximum(pair_counts, 1.0), 0.0)
    n_unique = float((counts > 0).sum())
    loss = 1.0 - avg.sum() / n_unique
    return np.float32(loss)


def _run(feature, pred, trace=False, **spmd_kwargs):
    from concourse.bass_utils import run_bass_kernel_spmd

    nc = _get_nc()
    in_maps = _make_in_maps(feature, pred)
    res = run_bass_kernel_spmd(
        nc, in_maps, core_ids=list(range(NCORES)), trace=trace, **spmd_kwargs
    )
    partials = [r["out"] for r in res.results]
    return _finish(partials, pred), res


def kernel(feature, pred, num_classes):
    assert int(num_classes) == C
    loss, _ = _run(feature, pred, trace=False)
    return loss
